# revision 14
# baseline (speedup 1.0000x reference)
"""Trainium2 Bass kernel for Bahdanau monotonic attention.

Math (per batch row):
    proj_key = memory @ W_k                      # [T, U]
    w[t]     = v . tanh(proj_q + proj_key[t])    # scores
    p        = sigmoid(w)
    C        = exclusive-cumprod(clip(1-p, 1e-20, 1))
    S        = cumsum(prev / clip(C, 1e-10, 1))
    align    = p * C * S
    ctx      = align @ memory                    # [E]

Distribution: batch rows are data-parallel across 8 NeuronCores (4 rows per
core), parameters replicated. The host hands each core `memory` pre-transposed
to [E, T] ("memT") so the E-contraction matmul gets unit-stride DMA; the
context reduction over T is then done on the vector/gpsimd/scalar engines from
the same resident tiles, so memory is read from HBM exactly once.

On-core layout: the 4 batch rows of all per-token vectors (w, p, C, S, ...)
live on SBUF partitions 0..3.  All compute-engine partition ranges start at 0
(hardware requires 32-aligned partition bases).  The score dot-product for all
4 rows accumulates into one [4, TC] PSUM tile using block-diagonal `v` tiles
(lhsT [128, 4] with v in column r, zeros elsewhere).  The alignment broadcast
(a[t] replicated across 128 partitions for the context reduction) is a f32r
matmul with a host-provided row-selector `eye4` [4, 512].

Numerics (validated offline against the jax reference):
  - fp32r matmuls: measured ~4e-5 relative on HW; score noise ~1e-4 moves
    outputs < 1e-4 of scale.
  - sigmoid via z = exp(-w); om = z/(1+z); p = 1-om: small RELATIVE error in
    om, which the log-free cumprod scan needs.  ACT uses only {Tanh, Exp,
    Copy} — one activation table, zero reload cost.
  - C is a direct fp32 cumprod scan (tensor_tensor_scan op0=mult), matching
    exp(cumsum(log(...))) to ~1e-5 relative.
  - 1/x via 2-ULP approx reciprocal (fast + one Newton step).
"""

import numpy as np

import concourse.bass as bass
import concourse.tile as tile
from concourse import bacc, mybir
from concourse.bass_utils import run_bass_kernel_spmd

F32 = mybir.dt.float32
F32R = mybir.dt.float32r
AF = mybir.ActivationFunctionType
ALU = mybir.AluOpType

# Problem constants (full problem; per-core shapes derived below).
N_FULL, T_DIM, E_DIM, D_DIM, U_DIM = 32, 2048, 512, 1024, 256
N_CORES = 8


def build_nc(R=4, T=2048, E=512, D=1024, U=256, TC=512, n_ctx_dve=9,
             memt_bufs=6):
    """Build the per-core Bass program."""
    NE, NU, ND, NT = E // 128, U // 128, D // 128, T // TC
    assert E % 128 == 0 and U % 128 == 0 and D % 128 == 0 and T % TC == 0
    assert R <= 4

    nc = bacc.Bacc("TRN2", target_bir_lowering=False, debug=False)

    memt_d = nc.dram_tensor("memT", [R, E, T], F32, kind="ExternalInput").ap()
    prev_d = nc.dram_tensor("prev", [R, T], F32, kind="ExternalInput").ap()
    q_d = nc.dram_tensor("queries", [R, 1, D], F32, kind="ExternalInput").ap()
    wq_d = nc.dram_tensor("W_q", [D, U], F32, kind="ExternalInput").ap()
    wk_d = nc.dram_tensor("W_k", [E, U], F32, kind="ExternalInput").ap()
    vblk_d = nc.dram_tensor("vblk", [128, U // 128, R, R], F32,
                            kind="ExternalInput").ap()
    eye_d = nc.dram_tensor("eye4", [R, R * 128], F32, kind="ExternalInput").ap()
    ctx_d = nc.dram_tensor("contexts", [R, E], F32, kind="ExternalOutput").ap()
    al_d = nc.dram_tensor("alignments", [R, T], F32, kind="ExternalOutput").ap()

    rsl = slice(0, R)

    with tile.TileContext(nc) as tc:
        with (
            tc.tile_pool(name="const", bufs=1) as const,
            tc.tile_pool(name="memt", bufs=memt_bufs) as memtp,
            tc.tile_pool(name="tanh", bufs=6) as tanhp,
            tc.tile_pool(name="sops", bufs=2) as sops,
            tc.tile_pool(name="arep", bufs=4) as arepp,
            tc.tile_pool(name="scr", bufs=4) as scrp,
            tc.tile_pool(name="pproj", bufs=3, space="PSUM") as pproj,
            tc.tile_pool(name="ppw", bufs=2, space="PSUM") as ppw,
            tc.tile_pool(name="parep", bufs=2, space="PSUM") as parep,
        ):
            # ---- constants / parameters -------------------------------
            wk_sb = []
            for ec in range(NE):
                wt = const.tile([128, U], F32R, tag=f"wk{ec}")
                nc.sync.dma_start(out=wt, in_=wk_d[ec * 128:(ec + 1) * 128, :].bitcast(F32R))
                wk_sb.append(wt)
            wq_sb = []
            for dc in range(ND):
                wt = const.tile([128, U], F32R, tag=f"wq{dc}")
                nc.sync.dma_start(out=wt, in_=wq_d[dc * 128:(dc + 1) * 128, :].bitcast(F32R))
                wq_sb.append(wt)
            # block-diagonal v (host-built): vcol[p, uh, r, j] = v[uh*128+p]
            # iff j == r, else 0
            vcol = const.tile([128, NU, R, R], F32R, tag="vcol")
            nc.sync.dma_start(out=vcol, in_=vblk_d.bitcast(F32R))
            eye_sb = const.tile([R, R * 128], F32R, tag="eye")
            nc.sync.dma_start(out=eye_sb, in_=eye_d.bitcast(F32R))
            qsb = const.tile([128, R * ND], F32R, tag="qsb")
            for r in range(R):
                nc.sync.dma_start(
                    out=qsb[:, r * ND:(r + 1) * ND],
                    in_=q_d[r, 0, :].rearrange("(a p) -> p a", p=128).bitcast(F32R))
            prev_sb = const.tile([R, T], F32, tag="prev")
            nc.sync.dma_start(out=prev_sb, in_=prev_d[:, :])

            # big per-token tiles; Cx/S have one extra leading column
            Cx = const.tile([R, T + 4], F32, tag="Cx")
            Ssc = const.tile([R, T + 4], F32, tag="Ssc")
            a_all = const.tile([R, T], F32R, tag="a_all")
            parts = const.tile([128, R * NE, NT], F32, tag="parts")
            ctxrow = const.tile([128, R * NE], F32, tag="ctxrow")
            nc.vector.memset(Cx[:, 0:1], 1.0)
            nc.vector.memset(Ssc[:, 0:1], 0.0)

            # ---- proj_q = q @ W_q, transposed into per-partition bias --
            # batched over rows: rhs [128, R] (fp32r needs moving dim > 1)
            projq_sb = const.tile([128, R * NU], F32, tag="projq")
            qsb_by_dc = qsb.rearrange("p (r a) -> p a r", a=ND)
            pq_by_uh = projq_sb.rearrange("p (r u) -> p u r", u=NU)
            for uh in range(NU):
                pq = ppw.tile([128, R], F32, tag="pw")
                for dc in range(ND):
                    nc.tensor.matmul(
                        pq,
                        lhsT=wq_sb[dc][:, uh * 128:(uh + 1) * 128],
                        rhs=qsb_by_dc[:, dc, :],
                        start=(dc == 0), stop=(dc == ND - 1))
                nc.scalar.copy(out=pq_by_uh[:, uh, :], in_=pq)

            # ---- main loop over token chunks --------------------------
            for g in range(NT):
                gsl = slice(g * TC, (g + 1) * TC)
                gsl1 = slice(g * TC + 1, (g + 1) * TC + 1)

                # memT tiles for this chunk: [128(e), NE, TC] per row
                memt = []
                for r in range(R):
                    mt = memtp.tile([128, NE, TC], F32R, tag="memt")
                    nc.sync.dma_start(
                        out=mt,
                        in_=memt_d[r].rearrange("(ec p) t -> p ec t", p=128)
                        [:, :, gsl].bitcast(F32R))
                    memt.append(mt)

                # scores: proj -> tanh -> blockdiag-v dot into pw[0:R, :]
                pw = ppw.tile([R, TC], F32, tag="pw")
                first = True
                for r in range(R):
                    for uh in range(NU):
                        pp = pproj.tile([128, TC], F32, tag="proj")
                        for ec in range(NE):
                            nc.tensor.matmul(
                                pp,
                                lhsT=wk_sb[ec][:, uh * 128:(uh + 1) * 128],
                                rhs=memt[r][:, ec, :],
                                start=(ec == 0), stop=(ec == NE - 1))
                        th = tanhp.tile([128, TC], F32R, tag="tanh")
                        nc.scalar.activation(
                            out=th, in_=pp, func=AF.Tanh,
                            bias=projq_sb[:, r * NU + uh: r * NU + uh + 1],
                            scale=1.0)
                        nc.tensor.matmul(
                            pw, lhsT=vcol[:, uh, r, :], rhs=th,
                            start=first,
                            stop=(r == R - 1 and uh == NU - 1))
                        first = False

                # sigmoid via exp: z = exp(-max(w,-85)); om = z/(1+z)
                wcl = sops.tile([R, TC], F32, tag="wcl")
                nc.vector.tensor_scalar(out=wcl, in0=pw[rsl, :],
                                        scalar1=-85.0, scalar2=None, op0=ALU.max)
                z = sops.tile([R, TC], F32, tag="z")
                nc.scalar.activation(out=z, in_=wcl, func=AF.Exp, scale=-1.0)
                den = sops.tile([R, TC], F32, tag="den")
                nc.gpsimd.tensor_scalar(out=den, in0=z, scalar1=1.0,
                                        scalar2=None, op0=ALU.add)
                rden = sops.tile([R, TC], F32, tag="rden")
                rscr = sops.tile([R, TC], F32, tag="rscr")
                nc.vector.reciprocal_approx_accurate(out=rden, in_=den, scratch=rscr)
                om = sops.tile([R, TC], F32, tag="om")
                nc.gpsimd.tensor_tensor(out=om, in0=z, in1=rden, op=ALU.mult)
                p_t = sops.tile([R, TC], F32, tag="p")
                nc.gpsimd.tensor_scalar(out=p_t, in0=om, scalar1=-1.0,
                                        scalar2=1.0, op0=ALU.mult, op1=ALU.add)
                omc = sops.tile([R, TC], F32, tag="omc")
                nc.gpsimd.tensor_scalar(out=omc, in0=om, scalar1=1e-20,
                                        scalar2=1.0, op0=ALU.max, op1=ALU.min)

                # C: inclusive cumprod written shifted by one -> Cx[:, 1+gsl]
                nc.vector.tensor_tensor_scan(
                    out=Cx[:, gsl1], data0=omc, data1=omc,
                    initial=Cx[:, g * TC: g * TC + 1],
                    op0=ALU.mult, op1=ALU.bypass)
                # ratio = prev / max(C, 1e-10)
                Cc = sops.tile([R, TC], F32, tag="Cc")
                nc.gpsimd.tensor_scalar(out=Cc, in0=Cx[:, gsl], scalar1=1e-10,
                                        scalar2=None, op0=ALU.max)
                rC = sops.tile([R, TC], F32, tag="rC")
                rscr2 = sops.tile([R, TC], F32, tag="rscr2")
                nc.vector.reciprocal_approx_accurate(out=rC, in_=Cc, scratch=rscr2)
                ratio = sops.tile([R, TC], F32, tag="ratio")
                nc.vector.tensor_tensor(out=ratio, in0=rC, in1=prev_sb[:, gsl],
                                        op=ALU.mult)
                # S: inclusive cumsum (shifted layout like Cx)
                nc.vector.tensor_tensor_scan(
                    out=Ssc[:, gsl1], data0=ratio, data1=ratio,
                    initial=Ssc[:, g * TC: g * TC + 1],
                    op0=ALU.add, op1=ALU.bypass)
                # a = p * C * S  (written as f32r so the broadcast matmul
                # can consume it; alignments DMA reads the same bits)
                pc = sops.tile([R, TC], F32, tag="pc")
                nc.gpsimd.tensor_tensor(out=pc, in0=p_t, in1=Cx[:, gsl],
                                        op=ALU.mult)
                nc.vector.tensor_tensor(out=a_all[:, gsl],
                                        in0=pc, in1=Ssc[:, gsl1], op=ALU.mult)
                nc.sync.dma_start(out=al_d[:, gsl], in_=a_all[:, gsl].bitcast(F32))

                # contexts partial: ctx[e] += sum_t a[t] * memT[e, t]
                for r in range(R):
                    arp = parep.tile([128, TC], F32, tag="arep")
                    nc.tensor.matmul(
                        arp, lhsT=eye_sb[:, r * 128:(r + 1) * 128],
                        rhs=a_all[:, gsl], start=True, stop=True)
                    ar = arepp.tile([128, TC], F32, tag="arep")
                    nc.scalar.copy(out=ar, in_=arp)
                    for ec in range(NE):
                        idx = r * NE + ec
                        if (idx * 16) // (R * NE) < n_ctx_dve:
                            scr = scrp.tile([128, TC], F32, tag="scrv")
                            nc.vector.scalar_tensor_tensor(
                                out=scr, in0=memt[r][:, ec, :].bitcast(F32),
                                scalar=1.0, in1=ar, op0=ALU.mult, op1=ALU.mult,
                                accum_out=parts[:, idx, g:g + 1])
                        else:
                            scr = scrp.tile([128, TC], F32, tag="scrg")
                            nc.gpsimd.tensor_tensor(
                                out=scr, in0=memt[r][:, ec, :].bitcast(F32),
                                in1=ar, op=ALU.mult)
                            nc.scalar.activation(
                                out=scr, in_=scr, func=AF.Copy,
                                accum_out=parts[:, idx, g:g + 1])
                    # hint: memt[r] fully consumed after this chunk's ctx ops

            # ---- finalize contexts ------------------------------------
            for r in range(R):
                for ec in range(NE):
                    idx = r * NE + ec
                    nc.vector.tensor_reduce(
                        out=ctxrow[:, idx:idx + 1], in_=parts[:, idx, :],
                        axis=mybir.AxisListType.X, op=ALU.add)
                nc.sync.dma_start(
                    out=ctx_d[r].rearrange("(ec p) -> p ec", p=128),
                    in_=ctxrow[:, r * NE:(r + 1) * NE])

    nc.compile()
    return nc


_NC_CACHE = {}


def _get_nc():
    key = "full"
    if key not in _NC_CACHE:
        _NC_CACHE[key] = build_nc(R=N_FULL // N_CORES, T=T_DIM, E=E_DIM,
                                  D=D_DIM, U=U_DIM)
    return _NC_CACHE[key]


def _make_eye4(R):
    eye = np.zeros((R, R * 128), dtype=np.float32)
    for r in range(R):
        eye[r, r * 128:(r + 1) * 128] = 1.0
    return eye


def _make_vblk(v, R):
    U = v.shape[0]
    NU = U // 128
    vblk = np.zeros((128, NU, R, R), dtype=np.float32)
    for uh in range(NU):
        for r in range(R):
            vblk[:, uh, r, r] = v[uh * 128:(uh + 1) * 128]
    return vblk


def kernel(queries, previous_alignments, memory, W_q, W_k, v):
    queries = np.ascontiguousarray(np.asarray(queries, dtype=np.float32))
    prev = np.ascontiguousarray(np.asarray(previous_alignments, dtype=np.float32))
    memory = np.asarray(memory, dtype=np.float32)
    W_q = np.ascontiguousarray(np.asarray(W_q, dtype=np.float32))
    W_k = np.ascontiguousarray(np.asarray(W_k, dtype=np.float32))
    v = np.ascontiguousarray(np.asarray(v, dtype=np.float32))

    memT = np.ascontiguousarray(memory.transpose(0, 2, 1))  # [N, E, T]
    R = N_FULL // N_CORES
    eye4 = _make_eye4(R)
    vblk = _make_vblk(v, R)

    nc = _get_nc()
    in_maps = []
    for c in range(N_CORES):
        sl = slice(c * R, (c + 1) * R)
        in_maps.append({
            "memT": memT[sl], "prev": prev[sl], "queries": queries[sl],
            "W_q": W_q, "W_k": W_k, "vblk": vblk, "eye4": eye4,
        })
    res = run_bass_kernel_spmd(nc, in_maps, core_ids=list(range(N_CORES)))
    contexts = np.concatenate([res.results[c]["contexts"] for c in range(N_CORES)], axis=0)
    alignments = np.concatenate([res.results[c]["alignments"] for c in range(N_CORES)], axis=0)
    return (contexts, alignments)


# revision 23
# speedup vs baseline: 1.0277x; 1.0277x over previous
"""Trainium2 Bass kernel for Bahdanau monotonic attention.

Math (per batch row):
    proj_key = memory @ W_k                      # [T, U]
    w[t]     = v . tanh(proj_q + proj_key[t])    # scores
    p        = sigmoid(w)
    C        = exclusive-cumprod(clip(1-p, 1e-20, 1))
    S        = cumsum(prev / clip(C, 1e-10, 1))
    align    = p * C * S
    ctx      = align @ memory                    # [E]

Distribution: batch rows are data-parallel across 8 NeuronCores (4 rows per
core), parameters replicated.  The host hands each core `memory` pre-
transposed to [E, T] ("memT") so the E-contraction matmul gets unit-stride
DMA; the context reduction over T runs on the vector/gpsimd/scalar engines
from the same resident tiles, so memory is read from HBM exactly once.

Schedule (per 512-token chunk g): memT DMAs (sync ring) -> proj matmuls +
tanh -> score dot (block-diagonal v, all 4 rows into one [4,TC] PSUM tile)
-> context reduction for chunk g-1 (software-pipelined one chunk behind so
the PE stream never waits on the scan chain) -> sigmoid/cumprod/cumsum scan
chain for chunk g.  Parameters load as single consolidated DMAs on the
scalar HWDGE ring so the sync ring is free for memT from cycle 0.

On-core layout: the 4 batch rows of per-token vectors live on partitions
0..3 (compute partition ranges must start 32-aligned, so row-r-only ops are
avoided entirely).  The alignment broadcast (a[t] replicated across 128
partitions) is an fp32r matmul with a host-provided row-selector `eye4`.

Numerics (validated offline + on HW):
  - fp32r matmuls (~4e-5 relative on HW): full PE rate for moving dim >= 256.
  - sigmoid via z = exp(-w); om = z/(1+z); p = 1-om: small RELATIVE error in
    om, which the log-free cumprod needs.  ACT uses only {Tanh, Exp, Copy} —
    one activation table, zero reload cost.
  - C is a direct fp32 cumprod scan (tensor_tensor_scan op0=mult).
  - 1/x via the ~51-ULP approx reciprocal (well within the noise floor).
Full-size HW check vs the jax reference: absmax-rel ~9e-4.
"""

import numpy as np

import concourse.bass as bass
import concourse.tile as tile
from concourse import bacc, mybir
from concourse.bass_utils import run_bass_kernel_spmd

F32 = mybir.dt.float32
F32R = mybir.dt.float32r
AF = mybir.ActivationFunctionType
ALU = mybir.AluOpType

N_FULL, T_DIM, E_DIM, D_DIM, U_DIM = 32, 2048, 512, 1024, 256
N_CORES = 8


def build_nc(R=4, T=2048, E=512, D=1024, U=256, TC=512, n_ctx_dve=10,
             memt_bufs=10, sops_bufs=1, tanh_bufs=8):
    NE, NU, ND, NT = E // 128, U // 128, D // 128, T // TC
    assert E % 128 == 0 and U % 128 == 0 and D % 128 == 0 and T % TC == 0
    assert R <= 4

    nc = bacc.Bacc("TRN2", target_bir_lowering=False, debug=False)

    memt_d = nc.dram_tensor("memT", [R, E, T], F32, kind="ExternalInput").ap()
    prev_d = nc.dram_tensor("prev", [R, T], F32, kind="ExternalInput").ap()
    q_d = nc.dram_tensor("queries", [R, 1, D], F32, kind="ExternalInput").ap()
    wq_d = nc.dram_tensor("W_q", [D, U], F32, kind="ExternalInput").ap()
    wk_d = nc.dram_tensor("W_k", [E, U], F32, kind="ExternalInput").ap()
    vblk_d = nc.dram_tensor("vblk", [128, U // 128, R, R], F32,
                            kind="ExternalInput").ap()
    eye_d = nc.dram_tensor("eye4", [R, R * 128], F32, kind="ExternalInput").ap()
    ctx_d = nc.dram_tensor("contexts", [R, E], F32, kind="ExternalOutput").ap()
    al_d = nc.dram_tensor("alignments", [R, T], F32, kind="ExternalOutput").ap()

    with tile.TileContext(nc) as tc:
        with (
            tc.tile_pool(name="const", bufs=1) as const,
            tc.tile_pool(name="memt", bufs=memt_bufs) as memtp,
            tc.tile_pool(name="tanh", bufs=tanh_bufs) as tanhp,
            tc.tile_pool(name="sops", bufs=sops_bufs) as sops,
            tc.tile_pool(name="arep", bufs=3) as arepp,
            tc.tile_pool(name="scr", bufs=2) as scrp,
            tc.tile_pool(name="pproj", bufs=3, space="PSUM") as pproj,
            tc.tile_pool(name="ppw", bufs=3, space="PSUM") as ppw,
            tc.tile_pool(name="parep", bufs=2, space="PSUM") as parep,
        ):
            # ---- first chunk's memT DMAs go out before anything else ----
            def load_memt(g):
                gsl = slice(g * TC, (g + 1) * TC)
                tiles = []
                for r in range(R):
                    mt = memtp.tile([128, NE, TC], F32R, tag="memt")
                    nc.sync.dma_start(
                        out=mt,
                        in_=memt_d[r].rearrange("(ec p) t -> p ec t", p=128)
                        [:, :, gsl].bitcast(F32R))
                    tiles.append(mt)
                return tiles

            memt_cur = load_memt(0)

            # ---- parameters: consolidated DMAs on the scalar ring -------
            # (ordered so proj_q's inputs land first)
            wq_sb = const.tile([128, ND, U], F32R, tag="wq")
            nc.scalar.dma_start(
                out=wq_sb, in_=wq_d.rearrange("(dc p) u -> p dc u", p=128).bitcast(F32R))
            qsb = const.tile([128, R * ND], F32R, tag="qsb")
            for r in range(R):
                nc.scalar.dma_start(
                    out=qsb[:, r * ND:(r + 1) * ND],
                    in_=q_d[r, 0, :].rearrange("(a p) -> p a", p=128).bitcast(F32R))
            wk_sb = const.tile([128, NE, U], F32R, tag="wk")
            nc.scalar.dma_start(
                out=wk_sb, in_=wk_d.rearrange("(ec p) u -> p ec u", p=128).bitcast(F32R))
            vcol = const.tile([128, NU, R, R], F32R, tag="vcol")
            nc.scalar.dma_start(out=vcol, in_=vblk_d.bitcast(F32R))
            eye_sb = const.tile([R, R * 128], F32R, tag="eye")
            nc.scalar.dma_start(out=eye_sb, in_=eye_d.bitcast(F32R))
            prev_sb = const.tile([R, T], F32, tag="prev")
            nc.scalar.dma_start(out=prev_sb, in_=prev_d[:, :])

            Cx = const.tile([R, T + 4], F32, tag="Cx")
            Ssc = const.tile([R, T + 4], F32, tag="Ssc")
            a_all = const.tile([R, T], F32R, tag="a_all")
            parts = []
            for i in range(R * NE):
                pt = const.tile([128, NT], F32, tag=f"parts{i}")
                parts.append(pt)
            ctxrow = const.tile([128, R * NE], F32, tag="ctxrow")
            eps_sb = const.tile([R, TC], F32, tag="eps")
            nc.vector.memset(eps_sb, 1e-20)
            nc.vector.memset(Cx[:, 0:1], 1.0)
            nc.vector.memset(Ssc[:, 0:1], 0.0)

            # ---- proj_q = q @ W_q (batched over rows; N=R) --------------
            projq_sb = const.tile([128, R * NU], F32, tag="projq")
            qsb_by_dc = qsb.rearrange("p (r a) -> p a r", a=ND)
            pq_by_uh = projq_sb.rearrange("p (r u) -> p u r", u=NU)
            with tc.high_priority():
                for uh in range(NU):
                    pq = ppw.tile([128, R], F32, tag="pw")
                    for dc in range(ND):
                        nc.tensor.matmul(
                            pq, lhsT=wq_sb[:, dc, uh * 128:(uh + 1) * 128],
                            rhs=qsb_by_dc[:, dc, :],
                            start=(dc == 0), stop=(dc == ND - 1))
                    nc.scalar.copy(out=pq_by_uh[:, uh, :], in_=pq)

            # ---- context reduction for one chunk (pipelined 1 behind) ---
            def emit_ctx(g, memt_tiles, n_dve=None):
                if n_dve is None:
                    n_dve = n_ctx_dve
                gsl = slice(g * TC, (g + 1) * TC)
                for r in range(R):
                    arp = parep.tile([128, TC], F32, tag="arep")
                    nc.tensor.matmul(
                        arp, lhsT=eye_sb[:, r * 128:(r + 1) * 128],
                        rhs=a_all[:, gsl], start=True, stop=True)
                    ar_sb = None
                    for ec in range(NE):
                        idx = r * NE + ec
                        if idx < n_dve:
                            scr = scrp.tile([128, TC], F32, tag="scrv")
                            nc.vector.scalar_tensor_tensor(
                                out=scr, in0=memt_tiles[r][:, ec, :].bitcast(F32),
                                scalar=1.0, in1=arp, op0=ALU.mult, op1=ALU.mult,
                                accum_out=parts[idx][:, g:g + 1])
                        else:
                            if ar_sb is None:
                                ar_sb = arepp.tile([128, TC], F32, tag="arep")
                                nc.scalar.copy(out=ar_sb, in_=arp)
                            scr = scrp.tile([128, TC], F32, tag="scrg")
                            nc.gpsimd.tensor_tensor(
                                out=scr, in0=memt_tiles[r][:, ec, :].bitcast(F32),
                                in1=ar_sb, op=ALU.mult)
                            nc.scalar.activation(
                                out=scr, in_=scr, func=AF.Copy,
                                accum_out=parts[idx][:, g:g + 1])

            # ---- main loop ----------------------------------------------
            memt_prev = None
            for g in range(NT):
                gsl = slice(g * TC, (g + 1) * TC)
                gsl1 = slice(g * TC + 1, (g + 1) * TC + 1)
                memt = memt_cur
                if g + 1 < NT:
                    memt_next = load_memt(g + 1)

                # scores: proj -> tanh (phase A), then dots (phase B)
                ths = []
                for r in range(R):
                    for uh in range(NU):
                        pp = pproj.tile([128, TC], F32, tag="proj")
                        for ec in range(NE):
                            nc.tensor.matmul(
                                pp, lhsT=wk_sb[:, ec, uh * 128:(uh + 1) * 128],
                                rhs=memt[r][:, ec, :],
                                start=(ec == 0), stop=(ec == NE - 1))
                        th = tanhp.tile([128, TC], F32R, tag="tanh")
                        nc.scalar.activation(
                            out=th, in_=pp, func=AF.Tanh,
                            bias=projq_sb[:, r * NU + uh: r * NU + uh + 1],
                            scale=1.0)
                        ths.append(th)
                pw = ppw.tile([R, TC], F32, tag="pw")
                k = 0
                for r in range(R):
                    for uh in range(NU):
                        nc.tensor.matmul(
                            pw, lhsT=vcol[:, uh, r, :], rhs=ths[k],
                            start=(k == 0), stop=(k == R * NU - 1))
                        k += 1

                # context reduction for the previous chunk (PE: 4 small
                # broadcast matmuls; DVE/gp/ACT: the multiplies+reductions)
                if memt_prev is not None:
                    emit_ctx(g - 1, memt_prev)

                # scan chain for this chunk.  z reads the score PSUM
                # directly; the 1e30 clamp keeps the reciprocal's bit-trick
                # seed in range even if exp(-w) saturates.
                z = sops.tile([R, TC], F32, tag="z")
                nc.scalar.activation(out=z, in_=pw[:, :], func=AF.Exp, scale=-1.0)
                den = sops.tile([R, TC], F32, tag="den")
                nc.vector.tensor_scalar(out=den, in0=z, scalar1=1e30,
                                        scalar2=1.0, op0=ALU.min, op1=ALU.add)
                rden = sops.tile([R, TC], F32, tag="rden")
                nc.vector.reciprocal_approx_fast(out=rden, in_=den)
                om = sops.tile([R, TC], F32, tag="om")
                nc.vector.tensor_tensor(out=om, in0=z, in1=rden, op=ALU.mult)
                p_t = sops.tile([R, TC], F32, tag="p")
                nc.vector.tensor_scalar(out=p_t, in0=om, scalar1=-1.0,
                                        scalar2=1.0, op0=ALU.mult, op1=ALU.add)
                nc.vector.tensor_tensor_scan(
                    out=Cx[:, gsl1], data0=om, data1=eps_sb[:, :],
                    initial=Cx[:, g * TC: g * TC + 1],
                    op0=ALU.mult, op1=ALU.max)
                Cc = sops.tile([R, TC], F32, tag="Cc")
                nc.vector.tensor_scalar(out=Cc, in0=Cx[:, gsl], scalar1=1e-10,
                                        scalar2=None, op0=ALU.max)
                rC = sops.tile([R, TC], F32, tag="rC")
                nc.vector.reciprocal_approx_fast(out=rC, in_=Cc)
                ratio = sops.tile([R, TC], F32, tag="ratio")
                nc.vector.tensor_tensor(out=ratio, in0=rC, in1=prev_sb[:, gsl],
                                        op=ALU.mult)
                nc.vector.tensor_tensor_scan(
                    out=Ssc[:, gsl1], data0=ratio, data1=ratio,
                    initial=Ssc[:, g * TC: g * TC + 1],
                    op0=ALU.add, op1=ALU.bypass)
                pc = sops.tile([R, TC], F32, tag="pc")
                nc.gpsimd.tensor_tensor(out=pc, in0=p_t, in1=Cx[:, gsl],
                                        op=ALU.mult)
                nc.vector.tensor_tensor(out=a_all[:, gsl],
                                        in0=pc, in1=Ssc[:, gsl1], op=ALU.mult)
                nc.scalar.dma_start(out=al_d[:, gsl], in_=a_all[:, gsl].bitcast(F32))

                memt_prev = memt
                if g + 1 < NT:
                    memt_cur = memt_next

            emit_ctx(NT - 1, memt_prev, n_dve=10)

            # ---- finalize contexts --------------------------------------
            for r in range(R):
                for ec in range(NE):
                    idx = r * NE + ec
                    nc.vector.tensor_reduce(
                        out=ctxrow[:, idx:idx + 1], in_=parts[idx][:, :],
                        axis=mybir.AxisListType.X, op=ALU.add)
                nc.scalar.dma_start(
                    out=ctx_d[r].rearrange("(ec p) -> p ec", p=128),
                    in_=ctxrow[:, r * NE:(r + 1) * NE])

    nc.compile()
    return nc


_NC_CACHE = {}


def _get_nc():
    key = "full"
    if key not in _NC_CACHE:
        _NC_CACHE[key] = build_nc(R=N_FULL // N_CORES, T=T_DIM, E=E_DIM,
                                  D=D_DIM, U=U_DIM)
    return _NC_CACHE[key]


def _make_eye4(R):
    eye = np.zeros((R, R * 128), dtype=np.float32)
    for r in range(R):
        eye[r, r * 128:(r + 1) * 128] = 1.0
    return eye


def _make_vblk(v, R):
    U = v.shape[0]
    NU = U // 128
    vblk = np.zeros((128, NU, R, R), dtype=np.float32)
    for uh in range(NU):
        for r in range(R):
            vblk[:, uh, r, r] = v[uh * 128:(uh + 1) * 128]
    return vblk


def kernel(queries, previous_alignments, memory, W_q, W_k, v):
    queries = np.ascontiguousarray(np.asarray(queries, dtype=np.float32))
    prev = np.ascontiguousarray(np.asarray(previous_alignments, dtype=np.float32))
    memory = np.asarray(memory, dtype=np.float32)
    W_q = np.ascontiguousarray(np.asarray(W_q, dtype=np.float32))
    W_k = np.ascontiguousarray(np.asarray(W_k, dtype=np.float32))
    v = np.ascontiguousarray(np.asarray(v, dtype=np.float32))

    memT = np.ascontiguousarray(memory.transpose(0, 2, 1))  # [N, E, T]
    R = N_FULL // N_CORES
    eye4 = _make_eye4(R)
    vblk = _make_vblk(v, R)

    nc = _get_nc()
    in_maps = []
    for c in range(N_CORES):
        sl = slice(c * R, (c + 1) * R)
        in_maps.append({
            "memT": memT[sl], "prev": prev[sl], "queries": queries[sl],
            "W_q": W_q, "W_k": W_k, "vblk": vblk, "eye4": eye4,
        })
    res = run_bass_kernel_spmd(nc, in_maps, core_ids=list(range(N_CORES)))
    contexts = np.concatenate([res.results[c]["contexts"] for c in range(N_CORES)], axis=0)
    alignments = np.concatenate([res.results[c]["alignments"] for c in range(N_CORES)], axis=0)
    return (contexts, alignments)


# revision 24
# speedup vs baseline: 21919.1502x; 21328.4787x over previous
"""Trainium2 Bass kernel for Bahdanau monotonic attention.

Math (per batch row):
    proj_key = memory @ W_k                      # [T, U]
    w[t]     = v . tanh(proj_q + proj_key[t])    # scores
    p        = sigmoid(w)
    C        = exclusive-cumprod(clip(1-p, 1e-20, 1))
    S        = cumsum(prev / clip(C, 1e-10, 1))
    align    = p * C * S
    ctx      = align @ memory                    # [E]

Distribution: batch rows are data-parallel across 8 NeuronCores (4 rows per
core), parameters replicated.  The host hands each core `memory` pre-
transposed to [E, T] ("memT") so the E-contraction matmul gets unit-stride
DMA; the context reduction over T runs on the vector/gpsimd/scalar engines
from the same resident tiles, so memory is read from HBM exactly once.

Schedule (per 512-token chunk g): memT DMAs (sync ring) -> proj matmuls +
tanh -> score dot (block-diagonal v, all 4 rows into one [4,TC] PSUM tile)
-> context reduction for chunk g-1 (software-pipelined one chunk behind so
the PE stream never waits on the scan chain) -> sigmoid/cumprod/cumsum scan
chain for chunk g.  Parameters load as single consolidated DMAs on the
scalar HWDGE ring so the sync ring is free for memT from cycle 0.

On-core layout: the 4 batch rows of per-token vectors live on partitions
0..3 (compute partition ranges must start 32-aligned, so row-r-only ops are
avoided entirely).  The alignment broadcast (a[t] replicated across 128
partitions) is an fp32r matmul with a host-provided row-selector `eye4`.

Numerics (validated offline + on HW):
  - fp32r matmuls (~4e-5 relative on HW): full PE rate for moving dim >= 256.
  - sigmoid via z = exp(-w); om = z/(1+z); p = 1-om: small RELATIVE error in
    om, which the log-free cumprod needs.  ACT uses only {Tanh, Exp, Copy} —
    one activation table, zero reload cost.
  - C is a direct fp32 cumprod scan (tensor_tensor_scan op0=mult).
  - 1/x via the ~51-ULP approx reciprocal (well within the noise floor).
Full-size HW check vs the jax reference: absmax-rel ~9e-4.
"""

import numpy as np

import concourse.bass as bass
import concourse.tile as tile
from concourse import bacc, mybir
from concourse.bass_utils import run_bass_kernel_spmd

F32 = mybir.dt.float32
F32R = mybir.dt.float32r
AF = mybir.ActivationFunctionType
ALU = mybir.AluOpType

N_FULL, T_DIM, E_DIM, D_DIM, U_DIM = 32, 2048, 512, 1024, 256
N_CORES = 8


def build_nc(R=4, T=2048, E=512, D=1024, U=256, TC=512, n_ctx_dve=11,
             memt_bufs=12, sops_bufs=1, tanh_bufs=8):
    NE, NU, ND, NT = E // 128, U // 128, D // 128, T // TC
    assert E % 128 == 0 and U % 128 == 0 and D % 128 == 0 and T % TC == 0
    assert R <= 4

    nc = bacc.Bacc("TRN2", target_bir_lowering=False, debug=False)

    memt_d = nc.dram_tensor("memT", [R, E, T], F32, kind="ExternalInput").ap()
    prev_d = nc.dram_tensor("prev", [R, T], F32, kind="ExternalInput").ap()
    q_d = nc.dram_tensor("queries", [R, 1, D], F32, kind="ExternalInput").ap()
    wq_d = nc.dram_tensor("W_q", [D, U], F32, kind="ExternalInput").ap()
    wk_d = nc.dram_tensor("W_k", [E, U], F32, kind="ExternalInput").ap()
    vblk_d = nc.dram_tensor("vblk", [128, U // 128, R, R], F32,
                            kind="ExternalInput").ap()
    eye_d = nc.dram_tensor("eye4", [R, R * 128], F32, kind="ExternalInput").ap()
    ctx_d = nc.dram_tensor("contexts", [R, E], F32, kind="ExternalOutput").ap()
    al_d = nc.dram_tensor("alignments", [R, T], F32, kind="ExternalOutput").ap()

    with tile.TileContext(nc) as tc:
        with (
            tc.tile_pool(name="const", bufs=1) as const,
            tc.tile_pool(name="memt", bufs=memt_bufs) as memtp,
            tc.tile_pool(name="tanh", bufs=tanh_bufs) as tanhp,
            tc.tile_pool(name="sops", bufs=sops_bufs) as sops,
            tc.tile_pool(name="arep", bufs=3) as arepp,
            tc.tile_pool(name="scr", bufs=2) as scrp,
            tc.tile_pool(name="pproj", bufs=3, space="PSUM") as pproj,
            tc.tile_pool(name="ppw", bufs=3, space="PSUM") as ppw,
            tc.tile_pool(name="parep", bufs=2, space="PSUM") as parep,
        ):
            # ---- first chunk's memT DMAs go out before anything else ----
            def load_memt(g):
                gsl = slice(g * TC, (g + 1) * TC)
                tiles = []
                for r in range(R):
                    mt = memtp.tile([128, NE, TC], F32R, tag="memt")
                    nc.sync.dma_start(
                        out=mt,
                        in_=memt_d[r].rearrange("(ec p) t -> p ec t", p=128)
                        [:, :, gsl].bitcast(F32R))
                    tiles.append(mt)
                return tiles

            memt_cur = load_memt(0)

            # ---- parameters: consolidated DMAs on the scalar ring -------
            # (ordered so proj_q's inputs land first)
            wq_sb = const.tile([128, ND, U], F32R, tag="wq")
            nc.scalar.dma_start(
                out=wq_sb, in_=wq_d.rearrange("(dc p) u -> p dc u", p=128).bitcast(F32R))
            qsb = const.tile([128, R * ND], F32R, tag="qsb")
            for r in range(R):
                nc.scalar.dma_start(
                    out=qsb[:, r * ND:(r + 1) * ND],
                    in_=q_d[r, 0, :].rearrange("(a p) -> p a", p=128).bitcast(F32R))
            wk_sb = const.tile([128, NE, U], F32R, tag="wk")
            nc.scalar.dma_start(
                out=wk_sb, in_=wk_d.rearrange("(ec p) u -> p ec u", p=128).bitcast(F32R))
            vcol = const.tile([128, NU, R, R], F32R, tag="vcol")
            nc.scalar.dma_start(out=vcol, in_=vblk_d.bitcast(F32R))
            eye_sb = const.tile([R, R * 128], F32R, tag="eye")
            nc.scalar.dma_start(out=eye_sb, in_=eye_d.bitcast(F32R))
            prev_sb = const.tile([R, T], F32, tag="prev")
            nc.scalar.dma_start(out=prev_sb, in_=prev_d[:, :])

            Cx = const.tile([R, T + 4], F32, tag="Cx")
            Ssc = const.tile([R, T + 4], F32, tag="Ssc")
            a_all = const.tile([R, T], F32R, tag="a_all")
            parts = []
            for i in range(R * NE):
                pt = const.tile([128, NT], F32, tag=f"parts{i}")
                parts.append(pt)
            ctxrow = const.tile([128, R * NE], F32, tag="ctxrow")
            eps_sb = const.tile([R, TC], F32, tag="eps")
            nc.vector.memset(eps_sb, 1e-20)
            nc.vector.memset(Cx[:, 0:1], 1.0)
            nc.vector.memset(Ssc[:, 0:1], 0.0)

            # ---- proj_q = q @ W_q (batched over rows; N=R) --------------
            projq_sb = const.tile([128, R * NU], F32, tag="projq")
            qsb_by_dc = qsb.rearrange("p (r a) -> p a r", a=ND)
            pq_by_uh = projq_sb.rearrange("p (r u) -> p u r", u=NU)
            with tc.high_priority():
                for uh in range(NU):
                    pq = ppw.tile([128, R], F32, tag="pw")
                    for dc in range(ND):
                        nc.tensor.matmul(
                            pq, lhsT=wq_sb[:, dc, uh * 128:(uh + 1) * 128],
                            rhs=qsb_by_dc[:, dc, :],
                            start=(dc == 0), stop=(dc == ND - 1))
                    nc.scalar.copy(out=pq_by_uh[:, uh, :], in_=pq)

            # ---- context reduction for one chunk (pipelined 1 behind) ---
            def emit_ctx(g, memt_tiles, n_dve=None):
                if n_dve is None:
                    n_dve = n_ctx_dve
                gsl = slice(g * TC, (g + 1) * TC)
                for r in range(R):
                    arp = parep.tile([128, TC], F32, tag="arep")
                    nc.tensor.matmul(
                        arp, lhsT=eye_sb[:, r * 128:(r + 1) * 128],
                        rhs=a_all[:, gsl], start=True, stop=True)
                    ar_sb = None
                    for ec in range(NE):
                        idx = r * NE + ec
                        if idx < n_dve:
                            scr = scrp.tile([128, TC], F32, tag="scrv")
                            nc.vector.scalar_tensor_tensor(
                                out=scr, in0=memt_tiles[r][:, ec, :].bitcast(F32),
                                scalar=1.0, in1=arp, op0=ALU.mult, op1=ALU.mult,
                                accum_out=parts[idx][:, g:g + 1])
                        else:
                            if ar_sb is None:
                                ar_sb = arepp.tile([128, TC], F32, tag="arep")
                                nc.scalar.copy(out=ar_sb, in_=arp)
                            scr = scrp.tile([128, TC], F32, tag="scrg")
                            nc.gpsimd.tensor_tensor(
                                out=scr, in0=memt_tiles[r][:, ec, :].bitcast(F32),
                                in1=ar_sb, op=ALU.mult)
                            nc.scalar.activation(
                                out=scr, in_=scr, func=AF.Copy,
                                accum_out=parts[idx][:, g:g + 1])

            # ---- main loop ----------------------------------------------
            memt_prev = None
            for g in range(NT):
                gsl = slice(g * TC, (g + 1) * TC)
                gsl1 = slice(g * TC + 1, (g + 1) * TC + 1)
                memt = memt_cur
                if g + 1 < NT:
                    memt_next = load_memt(g + 1)

                # scores: proj -> tanh (phase A), then dots (phase B)
                ths = []
                for r in range(R):
                    for uh in range(NU):
                        pp = pproj.tile([128, TC], F32, tag="proj")
                        for ec in range(NE):
                            nc.tensor.matmul(
                                pp, lhsT=wk_sb[:, ec, uh * 128:(uh + 1) * 128],
                                rhs=memt[r][:, ec, :],
                                start=(ec == 0), stop=(ec == NE - 1))
                        th = tanhp.tile([128, TC], F32R, tag="tanh")
                        nc.scalar.activation(
                            out=th, in_=pp, func=AF.Tanh,
                            bias=projq_sb[:, r * NU + uh: r * NU + uh + 1],
                            scale=1.0)
                        ths.append(th)
                pw = ppw.tile([R, TC], F32, tag="pw")
                k = 0
                for r in range(R):
                    for uh in range(NU):
                        nc.tensor.matmul(
                            pw, lhsT=vcol[:, uh, r, :], rhs=ths[k],
                            start=(k == 0), stop=(k == R * NU - 1))
                        k += 1

                # context reduction for the previous chunk (PE: 4 small
                # broadcast matmuls; DVE/gp/ACT: the multiplies+reductions)
                if memt_prev is not None:
                    emit_ctx(g - 1, memt_prev)

                # scan chain for this chunk.  z reads the score PSUM
                # directly; the 1e30 clamp keeps the reciprocal's bit-trick
                # seed in range even if exp(-w) saturates.
                z = sops.tile([R, TC], F32, tag="z")
                nc.scalar.activation(out=z, in_=pw[:, :], func=AF.Exp, scale=-1.0)
                den = sops.tile([R, TC], F32, tag="den")
                nc.vector.tensor_scalar(out=den, in0=z, scalar1=1e30,
                                        scalar2=1.0, op0=ALU.min, op1=ALU.add)
                rden = sops.tile([R, TC], F32, tag="rden")
                nc.vector.reciprocal_approx_fast(out=rden, in_=den)
                om = sops.tile([R, TC], F32, tag="om")
                nc.vector.tensor_tensor(out=om, in0=z, in1=rden, op=ALU.mult)
                p_t = sops.tile([R, TC], F32, tag="p")
                nc.vector.tensor_scalar(out=p_t, in0=om, scalar1=-1.0,
                                        scalar2=1.0, op0=ALU.mult, op1=ALU.add)
                nc.vector.tensor_tensor_scan(
                    out=Cx[:, gsl1], data0=om, data1=eps_sb[:, :],
                    initial=Cx[:, g * TC: g * TC + 1],
                    op0=ALU.mult, op1=ALU.max)
                Cc = sops.tile([R, TC], F32, tag="Cc")
                nc.vector.tensor_scalar(out=Cc, in0=Cx[:, gsl], scalar1=1e-10,
                                        scalar2=None, op0=ALU.max)
                rC = sops.tile([R, TC], F32, tag="rC")
                nc.vector.reciprocal_approx_fast(out=rC, in_=Cc)
                ratio = sops.tile([R, TC], F32, tag="ratio")
                nc.vector.tensor_tensor(out=ratio, in0=rC, in1=prev_sb[:, gsl],
                                        op=ALU.mult)
                nc.vector.tensor_tensor_scan(
                    out=Ssc[:, gsl1], data0=ratio, data1=ratio,
                    initial=Ssc[:, g * TC: g * TC + 1],
                    op0=ALU.add, op1=ALU.bypass)
                pc = sops.tile([R, TC], F32, tag="pc")
                nc.gpsimd.tensor_tensor(out=pc, in0=p_t, in1=Cx[:, gsl],
                                        op=ALU.mult)
                nc.vector.tensor_tensor(out=a_all[:, gsl],
                                        in0=pc, in1=Ssc[:, gsl1], op=ALU.mult)
                nc.scalar.dma_start(out=al_d[:, gsl], in_=a_all[:, gsl].bitcast(F32))

                memt_prev = memt
                if g + 1 < NT:
                    memt_cur = memt_next

            emit_ctx(NT - 1, memt_prev, n_dve=10)

            # ---- finalize contexts --------------------------------------
            for r in range(R):
                for ec in range(NE):
                    idx = r * NE + ec
                    nc.vector.tensor_reduce(
                        out=ctxrow[:, idx:idx + 1], in_=parts[idx][:, :],
                        axis=mybir.AxisListType.X, op=ALU.add)
                nc.scalar.dma_start(
                    out=ctx_d[r].rearrange("(ec p) -> p ec", p=128),
                    in_=ctxrow[:, r * NE:(r + 1) * NE])

    nc.compile()
    return nc


_NC_CACHE = {}


def _get_nc():
    key = "full"
    if key not in _NC_CACHE:
        _NC_CACHE[key] = build_nc(R=N_FULL // N_CORES, T=T_DIM, E=E_DIM,
                                  D=D_DIM, U=U_DIM)
    return _NC_CACHE[key]


def _make_eye4(R):
    eye = np.zeros((R, R * 128), dtype=np.float32)
    for r in range(R):
        eye[r, r * 128:(r + 1) * 128] = 1.0
    return eye


def _make_vblk(v, R):
    U = v.shape[0]
    NU = U // 128
    vblk = np.zeros((128, NU, R, R), dtype=np.float32)
    for uh in range(NU):
        for r in range(R):
            vblk[:, uh, r, r] = v[uh * 128:(uh + 1) * 128]
    return vblk


def kernel(queries, previous_alignments, memory, W_q, W_k, v):
    queries = np.ascontiguousarray(np.asarray(queries, dtype=np.float32))
    prev = np.ascontiguousarray(np.asarray(previous_alignments, dtype=np.float32))
    memory = np.asarray(memory, dtype=np.float32)
    W_q = np.ascontiguousarray(np.asarray(W_q, dtype=np.float32))
    W_k = np.ascontiguousarray(np.asarray(W_k, dtype=np.float32))
    v = np.ascontiguousarray(np.asarray(v, dtype=np.float32))

    memT = np.ascontiguousarray(memory.transpose(0, 2, 1))  # [N, E, T]
    R = N_FULL // N_CORES
    eye4 = _make_eye4(R)
    vblk = _make_vblk(v, R)

    nc = _get_nc()
    in_maps = []
    for c in range(N_CORES):
        sl = slice(c * R, (c + 1) * R)
        in_maps.append({
            "memT": memT[sl], "prev": prev[sl], "queries": queries[sl],
            "W_q": W_q, "W_k": W_k, "vblk": vblk, "eye4": eye4,
        })
    res = run_bass_kernel_spmd(nc, in_maps, core_ids=list(range(N_CORES)))
    contexts = np.concatenate([res.results[c]["contexts"] for c in range(N_CORES)], axis=0)
    alignments = np.concatenate([res.results[c]["alignments"] for c in range(N_CORES)], axis=0)
    return (contexts, alignments)


# revision 30
# speedup vs baseline: 22759.7966x; 1.0384x over previous
"""Trainium2 Bass kernel for Bahdanau monotonic attention.

Math (per batch row):
    proj_key = memory @ W_k                      # [T, U]
    w[t]     = v . tanh(proj_q + proj_key[t])    # scores
    p        = sigmoid(w)
    C        = exclusive-cumprod(clip(1-p, 1e-20, 1))
    S        = cumsum(prev / clip(C, 1e-10, 1))
    align    = p * C * S
    ctx      = align @ memory                    # [E]

Distribution: batch rows are data-parallel across 8 NeuronCores (4 rows per
core), parameters replicated.  The host hands each core `memory` pre-
transposed to [E, T] ("memT") so the E-contraction matmul gets unit-stride
DMA; the context reduction over T runs on the vector/gpsimd/scalar engines
from the same resident tiles, so memory is read from HBM exactly once.

Schedule (per 512-token chunk g): memT DMAs (sync ring) -> proj matmuls +
tanh -> score dot (block-diagonal v, all 4 rows into one [4,TC] PSUM tile)
-> context reduction for chunk g-1 (software-pipelined one chunk behind so
the PE stream never waits on the scan chain) -> sigmoid/cumprod/cumsum scan
chain for chunk g.  Parameters load as single consolidated DMAs on the
scalar HWDGE ring so the sync ring is free for memT from cycle 0.

On-core layout: the 4 batch rows of per-token vectors live on partitions
0..3 (compute partition ranges must start 32-aligned, so row-r-only ops are
avoided entirely).  The alignment broadcast (a[t] replicated across 128
partitions) is an fp32r matmul with a host-provided row-selector `eye4`.

Numerics (validated offline + on HW):
  - fp32r matmuls (~4e-5 relative on HW): full PE rate for moving dim >= 256.
  - sigmoid via z = exp(-w); om = z/(1+z); p = 1-om: small RELATIVE error in
    om, which the log-free cumprod needs.  ACT uses only {Tanh, Exp, Copy} —
    one activation table, zero reload cost.
  - C is a direct fp32 cumprod scan (tensor_tensor_scan op0=mult).
  - 1/x via the ~51-ULP approx reciprocal (well within the noise floor).
Full-size HW check vs the jax reference: absmax-rel ~9e-4.
"""

import numpy as np

import concourse.bass as bass
import concourse.tile as tile
from concourse import bacc, mybir
from concourse.bass_utils import run_bass_kernel_spmd

F32 = mybir.dt.float32
F32R = mybir.dt.float32r
AF = mybir.ActivationFunctionType
ALU = mybir.AluOpType

N_FULL, T_DIM, E_DIM, D_DIM, U_DIM = 32, 2048, 512, 1024, 256
N_CORES = 8


def build_nc(R=4, T=2048, E=512, D=1024, U=256, TC=512, n_ctx_dve=11,
             memt_bufs=13, sops_bufs=1, tanh_bufs=6):
    NE, NU, ND, NT = E // 128, U // 128, D // 128, T // TC
    assert E % 128 == 0 and U % 128 == 0 and D % 128 == 0 and T % TC == 0
    assert R <= 4

    nc = bacc.Bacc("TRN2", target_bir_lowering=False, debug=False)

    memt_d = nc.dram_tensor("memT", [R, E, T], F32, kind="ExternalInput").ap()
    prev_d = nc.dram_tensor("prev", [R, T], F32, kind="ExternalInput").ap()
    q_d = nc.dram_tensor("queries", [R, 1, D], F32, kind="ExternalInput").ap()
    wq_d = nc.dram_tensor("W_q", [D, U], F32, kind="ExternalInput").ap()
    wk_d = nc.dram_tensor("W_k", [E, U], F32, kind="ExternalInput").ap()
    vblk_d = nc.dram_tensor("vblk", [128, U // 128, R, R], F32,
                            kind="ExternalInput").ap()
    eye_d = nc.dram_tensor("eye4", [R, R * 128], F32, kind="ExternalInput").ap()
    ctx_d = nc.dram_tensor("contexts", [R, E], F32, kind="ExternalOutput").ap()
    al_d = nc.dram_tensor("alignments", [R, T], F32, kind="ExternalOutput").ap()

    with tile.TileContext(nc) as tc:
        with (
            tc.tile_pool(name="const", bufs=1) as const,
            tc.tile_pool(name="memt", bufs=memt_bufs) as memtp,
            tc.tile_pool(name="tanh", bufs=tanh_bufs) as tanhp,
            tc.tile_pool(name="sops", bufs=sops_bufs) as sops,
            tc.tile_pool(name="arep", bufs=3) as arepp,
            tc.tile_pool(name="scr", bufs=2) as scrp,
            tc.tile_pool(name="pproj", bufs=3, space="PSUM") as pproj,
            tc.tile_pool(name="ppw", bufs=3, space="PSUM") as ppw,
            tc.tile_pool(name="parep", bufs=2, space="PSUM") as parep,
        ):
            # ---- first chunk's memT DMAs go out before anything else ----
            def load_memt(g):
                gsl = slice(g * TC, (g + 1) * TC)
                tiles = []
                for r in range(R):
                    mt = memtp.tile([128, NE, TC], F32R, tag="memt")
                    nc.sync.dma_start(
                        out=mt,
                        in_=memt_d[r].rearrange("(ec p) t -> p ec t", p=128)
                        [:, :, gsl].bitcast(F32R))
                    tiles.append(mt)
                return tiles

            # ---- parameters first: proj's inputs gate the pipeline ------
            wq_sb = const.tile([128, ND, U], F32R, tag="wq")
            nc.scalar.dma_start(
                out=wq_sb, in_=wq_d.rearrange("(dc p) u -> p dc u", p=128).bitcast(F32R))
            wk_sb = const.tile([128, NE, U], F32R, tag="wk")
            nc.scalar.dma_start(
                out=wk_sb, in_=wk_d.rearrange("(ec p) u -> p ec u", p=128).bitcast(F32R))
            qsb = const.tile([128, R * ND], F32R, tag="qsb")
            for r in range(R):
                nc.scalar.dma_start(
                    out=qsb[:, r * ND:(r + 1) * ND],
                    in_=q_d[r, 0, :].rearrange("(a p) -> p a", p=128).bitcast(F32R))

            memt_cur = load_memt(0)

            vcol = const.tile([128, NU, R, R], F32R, tag="vcol")
            nc.scalar.dma_start(out=vcol, in_=vblk_d.bitcast(F32R))
            eye_sb = const.tile([R, R * 128], F32R, tag="eye")
            nc.scalar.dma_start(out=eye_sb, in_=eye_d.bitcast(F32R))
            prev_sb = const.tile([R, T], F32, tag="prev")
            nc.scalar.dma_start(out=prev_sb, in_=prev_d[:, :])

            Cx = const.tile([R, T + 4], F32, tag="Cx")
            Ssc = const.tile([R, T + 4], F32, tag="Ssc")
            a_all = const.tile([R, T], F32R, tag="a_all")
            parts = []
            for i in range(R * NE):
                pt = const.tile([128, NT], F32, tag=f"parts{i}")
                parts.append(pt)
            ctxrow = const.tile([128, R * NE], F32, tag="ctxrow")
            eps_sb = const.tile([R, TC], F32, tag="eps")
            nc.vector.memset(eps_sb, 1e-20)
            nc.vector.memset(Cx[:, 0:1], 1.0)
            nc.vector.memset(Ssc[:, 0:1], 0.0)

            # ---- proj_q = q @ W_q (batched over rows; N=R) --------------
            projq_sb = const.tile([128, R * NU], F32, tag="projq")
            qsb_by_dc = qsb.rearrange("p (r a) -> p a r", a=ND)
            pq_by_uh = projq_sb.rearrange("p (r u) -> p u r", u=NU)
            with tc.high_priority():
                for uh in range(NU):
                    pq = ppw.tile([128, R], F32, tag="pw")
                    for dc in range(ND):
                        nc.tensor.matmul(
                            pq, lhsT=wq_sb[:, dc, uh * 128:(uh + 1) * 128],
                            rhs=qsb_by_dc[:, dc, :],
                            start=(dc == 0), stop=(dc == ND - 1))
                    nc.scalar.copy(out=pq_by_uh[:, uh, :], in_=pq)

            # ---- context reduction for one chunk (pipelined 1 behind) ---
            def emit_ctx(g, memt_tiles, n_dve=None):
                if n_dve is None:
                    n_dve = n_ctx_dve
                gsl = slice(g * TC, (g + 1) * TC)
                for r in range(R):
                    arp = parep.tile([128, TC], F32, tag="arep")
                    nc.tensor.matmul(
                        arp, lhsT=eye_sb[:, r * 128:(r + 1) * 128],
                        rhs=a_all[:, gsl], start=True, stop=True)
                    ar_sb = None
                    for ec in range(NE):
                        idx = r * NE + ec
                        if idx < n_dve:
                            scr = scrp.tile([128, TC], F32, tag="scrv")
                            nc.vector.scalar_tensor_tensor(
                                out=scr, in0=memt_tiles[r][:, ec, :].bitcast(F32),
                                scalar=1.0, in1=arp, op0=ALU.mult, op1=ALU.mult,
                                accum_out=parts[idx][:, g:g + 1])
                        else:
                            if ar_sb is None:
                                ar_sb = arepp.tile([128, TC], F32, tag="arep")
                                nc.scalar.copy(out=ar_sb, in_=arp)
                            scr = scrp.tile([128, TC], F32, tag="scrg")
                            nc.gpsimd.tensor_tensor(
                                out=scr, in0=memt_tiles[r][:, ec, :].bitcast(F32),
                                in1=ar_sb, op=ALU.mult)
                            nc.scalar.activation(
                                out=scr, in_=scr, func=AF.Copy,
                                accum_out=parts[idx][:, g:g + 1])

            # ---- main loop ----------------------------------------------
            memt_prev = None
            for g in range(NT):
                gsl = slice(g * TC, (g + 1) * TC)
                gsl1 = slice(g * TC + 1, (g + 1) * TC + 1)
                memt = memt_cur
                if g + 1 < NT:
                    memt_next = load_memt(g + 1)

                # scores: proj -> tanh (phase A), then dots (phase B)
                ths = []
                for r in range(R):
                    for uh in range(NU):
                        pp = pproj.tile([128, TC], F32, tag="proj")
                        for ec in range(NE):
                            nc.tensor.matmul(
                                pp, lhsT=wk_sb[:, ec, uh * 128:(uh + 1) * 128],
                                rhs=memt[r][:, ec, :],
                                start=(ec == 0), stop=(ec == NE - 1))
                        th = tanhp.tile([128, TC], F32R, tag="tanh")
                        nc.scalar.activation(
                            out=th, in_=pp, func=AF.Tanh,
                            bias=projq_sb[:, r * NU + uh: r * NU + uh + 1],
                            scale=1.0)
                        ths.append(th)
                pw = ppw.tile([R, TC], F32, tag="pw")
                k = 0
                for r in range(R):
                    for uh in range(NU):
                        nc.tensor.matmul(
                            pw, lhsT=vcol[:, uh, r, :], rhs=ths[k],
                            start=(k == 0), stop=(k == R * NU - 1))
                        k += 1

                # context reduction for the previous chunk (PE: 4 small
                # broadcast matmuls; DVE/gp/ACT: the multiplies+reductions)
                if memt_prev is not None:
                    emit_ctx(g - 1, memt_prev,
                             n_dve=(n_ctx_dve if g - 1 < NT - 2 else 8))

                # scan chain for this chunk.  z reads the score PSUM
                # directly; the 1e30 clamp keeps the reciprocal's bit-trick
                # seed in range even if exp(-w) saturates.
                z = sops.tile([R, TC], F32, tag="z")
                nc.scalar.activation(out=z, in_=pw[:, :], func=AF.Exp, scale=-1.0)
                den = sops.tile([R, TC], F32, tag="den")
                nc.vector.tensor_scalar(out=den, in0=z, scalar1=1e30,
                                        scalar2=1.0, op0=ALU.min, op1=ALU.add)
                rden = sops.tile([R, TC], F32, tag="rden")
                nc.vector.reciprocal_approx_fast(out=rden, in_=den)
                om = sops.tile([R, TC], F32, tag="om")
                nc.vector.tensor_tensor(out=om, in0=z, in1=rden, op=ALU.mult)
                p_t = sops.tile([R, TC], F32, tag="p")
                nc.vector.tensor_scalar(out=p_t, in0=om, scalar1=-1.0,
                                        scalar2=1.0, op0=ALU.mult, op1=ALU.add)
                nc.vector.tensor_tensor_scan(
                    out=Cx[:, gsl1], data0=om, data1=eps_sb[:, :],
                    initial=Cx[:, g * TC: g * TC + 1],
                    op0=ALU.mult, op1=ALU.max)
                Cc = sops.tile([R, TC], F32, tag="Cc")
                nc.vector.tensor_scalar(out=Cc, in0=Cx[:, gsl], scalar1=1e-10,
                                        scalar2=None, op0=ALU.max)
                rC = sops.tile([R, TC], F32, tag="rC")
                nc.vector.reciprocal_approx_fast(out=rC, in_=Cc)
                ratio = sops.tile([R, TC], F32, tag="ratio")
                nc.vector.tensor_tensor(out=ratio, in0=rC, in1=prev_sb[:, gsl],
                                        op=ALU.mult)
                nc.vector.tensor_tensor_scan(
                    out=Ssc[:, gsl1], data0=ratio, data1=ratio,
                    initial=Ssc[:, g * TC: g * TC + 1],
                    op0=ALU.add, op1=ALU.bypass)
                pc = sops.tile([R, TC], F32, tag="pc")
                nc.gpsimd.tensor_tensor(out=pc, in0=p_t, in1=Cx[:, gsl],
                                        op=ALU.mult)
                nc.vector.tensor_tensor(out=a_all[:, gsl],
                                        in0=pc, in1=Ssc[:, gsl1], op=ALU.mult)
                nc.scalar.dma_start(out=al_d[:, gsl], in_=a_all[:, gsl].bitcast(F32))

                memt_prev = memt
                if g + 1 < NT:
                    memt_cur = memt_next

            emit_ctx(NT - 1, memt_prev, n_dve=8)

            # ---- finalize contexts --------------------------------------
            for r in range(R):
                for ec in range(NE):
                    idx = r * NE + ec
                    nc.vector.tensor_reduce(
                        out=ctxrow[:, idx:idx + 1], in_=parts[idx][:, :],
                        axis=mybir.AxisListType.X, op=ALU.add)
                nc.scalar.dma_start(
                    out=ctx_d[r].rearrange("(ec p) -> p ec", p=128),
                    in_=ctxrow[:, r * NE:(r + 1) * NE])

    nc.compile()
    return nc


_NC_CACHE = {}


def _get_nc():
    key = "full"
    if key not in _NC_CACHE:
        _NC_CACHE[key] = build_nc(R=N_FULL // N_CORES, T=T_DIM, E=E_DIM,
                                  D=D_DIM, U=U_DIM)
    return _NC_CACHE[key]


def _make_eye4(R):
    eye = np.zeros((R, R * 128), dtype=np.float32)
    for r in range(R):
        eye[r, r * 128:(r + 1) * 128] = 1.0
    return eye


def _make_vblk(v, R):
    U = v.shape[0]
    NU = U // 128
    vblk = np.zeros((128, NU, R, R), dtype=np.float32)
    for uh in range(NU):
        for r in range(R):
            vblk[:, uh, r, r] = v[uh * 128:(uh + 1) * 128]
    return vblk


def kernel(queries, previous_alignments, memory, W_q, W_k, v):
    queries = np.ascontiguousarray(np.asarray(queries, dtype=np.float32))
    prev = np.ascontiguousarray(np.asarray(previous_alignments, dtype=np.float32))
    memory = np.asarray(memory, dtype=np.float32)
    W_q = np.ascontiguousarray(np.asarray(W_q, dtype=np.float32))
    W_k = np.ascontiguousarray(np.asarray(W_k, dtype=np.float32))
    v = np.ascontiguousarray(np.asarray(v, dtype=np.float32))

    memT = np.ascontiguousarray(memory.transpose(0, 2, 1))  # [N, E, T]
    R = N_FULL // N_CORES
    eye4 = _make_eye4(R)
    vblk = _make_vblk(v, R)

    nc = _get_nc()
    in_maps = []
    for c in range(N_CORES):
        sl = slice(c * R, (c + 1) * R)
        in_maps.append({
            "memT": memT[sl], "prev": prev[sl], "queries": queries[sl],
            "W_q": W_q, "W_k": W_k, "vblk": vblk, "eye4": eye4,
        })
    res = run_bass_kernel_spmd(nc, in_maps, core_ids=list(range(N_CORES)))
    contexts = np.concatenate([res.results[c]["contexts"] for c in range(N_CORES)], axis=0)
    alignments = np.concatenate([res.results[c]["alignments"] for c in range(N_CORES)], axis=0)
    return (contexts, alignments)


# revision 32
# speedup vs baseline: 23427.5314x; 1.0293x over previous
"""Trainium2 Bass kernel for Bahdanau monotonic attention.

Math (per batch row):
    proj_key = memory @ W_k                      # [T, U]
    w[t]     = v . tanh(proj_q + proj_key[t])    # scores
    p        = sigmoid(w)
    C        = exclusive-cumprod(clip(1-p, 1e-20, 1))
    S        = cumsum(prev / clip(C, 1e-10, 1))
    align    = p * C * S
    ctx      = align @ memory                    # [E]

Distribution: batch rows are data-parallel across 8 NeuronCores (4 rows per
core), parameters replicated.  The host hands each core `memory` pre-
transposed to [E, T] ("memT") so the E-contraction matmul gets unit-stride
DMA; the context reduction over T runs on the vector/gpsimd/scalar engines
from the same resident tiles, so memory is read from HBM exactly once.

Schedule (per 512-token chunk g): memT DMAs (sync ring) -> proj matmuls +
tanh -> score dot (block-diagonal v, all 4 rows into one [4,TC] PSUM tile)
-> context reduction for chunk g-1 (software-pipelined one chunk behind so
the PE stream never waits on the scan chain) -> sigmoid/cumprod/cumsum scan
chain for chunk g.  Parameters load as single consolidated DMAs on the
scalar HWDGE ring so the sync ring is free for memT from cycle 0.

On-core layout: the 4 batch rows of per-token vectors live on partitions
0..3 (compute partition ranges must start 32-aligned, so row-r-only ops are
avoided entirely).  The alignment broadcast (a[t] replicated across 128
partitions) is an fp32r matmul with a host-provided row-selector `eye4`.

Numerics (validated offline + on HW):
  - fp32r matmuls (~4e-5 relative on HW): full PE rate for moving dim >= 256.
  - sigmoid via z = exp(-w); om = z/(1+z); p = 1-om: small RELATIVE error in
    om, which the log-free cumprod needs.  ACT uses only {Tanh, Exp, Copy} —
    one activation table, zero reload cost.
  - C is a direct fp32 cumprod scan (tensor_tensor_scan op0=mult).
  - 1/x via the ~51-ULP approx reciprocal (well within the noise floor).
Full-size HW check vs the jax reference: absmax-rel ~9e-4.
"""

import numpy as np

import concourse.bass as bass
import concourse.tile as tile
from concourse import bacc, mybir
from concourse.bass_utils import run_bass_kernel_spmd

F32 = mybir.dt.float32
F32R = mybir.dt.float32r
AF = mybir.ActivationFunctionType
ALU = mybir.AluOpType

N_FULL, T_DIM, E_DIM, D_DIM, U_DIM = 32, 2048, 512, 1024, 256
N_CORES = 8


def build_nc(R=4, T=2048, E=512, D=1024, U=256, TC=512, n_ctx_dve=11,
             memt_bufs=13, sops_bufs=1, tanh_bufs=6):
    NE, NU, ND, NT = E // 128, U // 128, D // 128, T // TC
    assert E % 128 == 0 and U % 128 == 0 and D % 128 == 0 and T % TC == 0
    assert R <= 4

    nc = bacc.Bacc("TRN2", target_bir_lowering=False, debug=False)

    memt_d = nc.dram_tensor("memT", [R, E, T], F32, kind="ExternalInput").ap()
    prev_d = nc.dram_tensor("prev", [R, T], F32, kind="ExternalInput").ap()
    q_d = nc.dram_tensor("queries", [R, 1, D], F32, kind="ExternalInput").ap()
    wq_d = nc.dram_tensor("W_q", [D, U], F32, kind="ExternalInput").ap()
    wk_d = nc.dram_tensor("W_k", [E, U], F32, kind="ExternalInput").ap()
    vblk_d = nc.dram_tensor("vblk", [128, U // 128, R, R], F32,
                            kind="ExternalInput").ap()
    eye_d = nc.dram_tensor("eye4", [R, R * 128], F32, kind="ExternalInput").ap()
    ctx_d = nc.dram_tensor("contexts", [R, E], F32, kind="ExternalOutput").ap()
    al_d = nc.dram_tensor("alignments", [R, T], F32, kind="ExternalOutput").ap()

    with tile.TileContext(nc) as tc:
        with (
            tc.tile_pool(name="const", bufs=1) as const,
            tc.tile_pool(name="memt", bufs=memt_bufs) as memtp,
            tc.tile_pool(name="tanh", bufs=tanh_bufs) as tanhp,
            tc.tile_pool(name="sops", bufs=sops_bufs) as sops,
            tc.tile_pool(name="arep", bufs=3) as arepp,
            tc.tile_pool(name="scr", bufs=2) as scrp,
            tc.tile_pool(name="pproj", bufs=3, space="PSUM") as pproj,
            tc.tile_pool(name="ppw", bufs=3, space="PSUM") as ppw,
            tc.tile_pool(name="parep", bufs=2, space="PSUM") as parep,
        ):
            # ---- first chunk's memT DMAs go out before anything else ----
            def load_memt(g):
                gsl = slice(g * TC, (g + 1) * TC)
                tiles = []
                for r in range(R):
                    mt = memtp.tile([128, NE, TC], F32R, tag="memt")
                    nc.sync.dma_start(
                        out=mt,
                        in_=memt_d[r].rearrange("(ec p) t -> p ec t", p=128)
                        [:, :, gsl].bitcast(F32R))
                    tiles.append(mt)
                return tiles

            # ---- parameters first: proj's inputs gate the pipeline ------
            wq_sb = const.tile([128, ND, U], F32R, tag="wq")
            nc.scalar.dma_start(
                out=wq_sb, in_=wq_d.rearrange("(dc p) u -> p dc u", p=128).bitcast(F32R))
            wk_sb = const.tile([128, NE, U], F32R, tag="wk")
            nc.scalar.dma_start(
                out=wk_sb, in_=wk_d.rearrange("(ec p) u -> p ec u", p=128).bitcast(F32R))
            qsb = const.tile([128, R * ND], F32R, tag="qsb")
            for r in range(R):
                nc.scalar.dma_start(
                    out=qsb[:, r * ND:(r + 1) * ND],
                    in_=q_d[r, 0, :].rearrange("(a p) -> p a", p=128).bitcast(F32R))

            memt_cur = load_memt(0)

            vcol = const.tile([128, NU, R, R], F32R, tag="vcol")
            nc.scalar.dma_start(out=vcol, in_=vblk_d.bitcast(F32R))
            eye_sb = const.tile([R, R * 128], F32R, tag="eye")
            nc.scalar.dma_start(out=eye_sb, in_=eye_d.bitcast(F32R))
            prev_sb = const.tile([R, T], F32, tag="prev")
            nc.scalar.dma_start(out=prev_sb, in_=prev_d[:, :])

            Cx = const.tile([R, T + 4], F32, tag="Cx")
            Ssc = const.tile([R, T + 4], F32, tag="Ssc")
            a_all = const.tile([R, T], F32R, tag="a_all")
            parts = []
            for i in range(R * NE):
                pt = const.tile([128, NT], F32, tag=f"parts{i}")
                parts.append(pt)
            ctxrow = const.tile([128, R * NE], F32, tag="ctxrow")
            eps_sb = const.tile([R, TC], F32, tag="eps")
            nc.vector.memset(eps_sb, 1e-20)
            nc.vector.memset(Cx[:, 0:1], 1.0)
            nc.vector.memset(Ssc[:, 0:1], 0.0)

            # ---- proj_q = q @ W_q (batched over rows; N=R) --------------
            projq_sb = const.tile([128, R * NU], F32, tag="projq")
            qsb_by_dc = qsb.rearrange("p (r a) -> p a r", a=ND)
            pq_by_uh = projq_sb.rearrange("p (r u) -> p u r", u=NU)
            with tc.high_priority():
                for uh in range(NU):
                    pq = ppw.tile([128, R], F32, tag="pw")
                    for dc in range(ND):
                        nc.tensor.matmul(
                            pq, lhsT=wq_sb[:, dc, uh * 128:(uh + 1) * 128],
                            rhs=qsb_by_dc[:, dc, :],
                            start=(dc == 0), stop=(dc == ND - 1))
                    nc.scalar.copy(out=pq_by_uh[:, uh, :], in_=pq)

            # ---- context reduction for one chunk (pipelined 1 behind) ---
            def emit_ctx(g, memt_tiles, n_dve=None):
                if n_dve is None:
                    n_dve = n_ctx_dve
                gsl = slice(g * TC, (g + 1) * TC)
                for r in range(R):
                    arp = parep.tile([128, TC], F32, tag="arep")
                    nc.tensor.matmul(
                        arp, lhsT=eye_sb[:, r * 128:(r + 1) * 128],
                        rhs=a_all[:, gsl], start=True, stop=True)
                    ar_sb = None
                    for ec in range(NE):
                        idx = r * NE + ec
                        if idx < n_dve:
                            scr = scrp.tile([128, TC], F32, tag="scrv")
                            nc.vector.scalar_tensor_tensor(
                                out=scr, in0=memt_tiles[r][:, ec, :].bitcast(F32),
                                scalar=1.0, in1=arp, op0=ALU.mult, op1=ALU.mult,
                                accum_out=parts[idx][:, g:g + 1])
                        else:
                            if ar_sb is None:
                                ar_sb = arepp.tile([128, TC], F32, tag="arep")
                                nc.scalar.copy(out=ar_sb, in_=arp)
                            scr = scrp.tile([128, TC], F32, tag="scrg")
                            nc.gpsimd.tensor_tensor(
                                out=scr, in0=memt_tiles[r][:, ec, :].bitcast(F32),
                                in1=ar_sb, op=ALU.mult)
                            nc.scalar.activation(
                                out=scr, in_=scr, func=AF.Copy,
                                accum_out=parts[idx][:, g:g + 1])

            # ---- main loop ----------------------------------------------
            memt_prev = None
            for g in range(NT):
                gsl = slice(g * TC, (g + 1) * TC)
                gsl1 = slice(g * TC + 1, (g + 1) * TC + 1)
                memt = memt_cur
                if g + 1 < NT:
                    memt_next = load_memt(g + 1)

                # scores: proj -> tanh (phase A), then dots (phase B)
                ths = []
                for r in range(R):
                    for uh in range(NU):
                        pp = pproj.tile([128, TC], F32, tag="proj")
                        for ec in range(NE):
                            nc.tensor.matmul(
                                pp, lhsT=wk_sb[:, ec, uh * 128:(uh + 1) * 128],
                                rhs=memt[r][:, ec, :],
                                start=(ec == 0), stop=(ec == NE - 1))
                        th = tanhp.tile([128, TC], F32R, tag="tanh")
                        nc.scalar.activation(
                            out=th, in_=pp, func=AF.Tanh,
                            bias=projq_sb[:, r * NU + uh: r * NU + uh + 1],
                            scale=1.0)
                        ths.append(th)
                pw = ppw.tile([R, TC], F32, tag="pw")
                k = 0
                for r in range(R):
                    for uh in range(NU):
                        nc.tensor.matmul(
                            pw, lhsT=vcol[:, uh, r, :], rhs=ths[k],
                            start=(k == 0), stop=(k == R * NU - 1))
                        k += 1

                # context reduction for the previous chunk (PE: 4 small
                # broadcast matmuls; DVE/gp/ACT: the multiplies+reductions)
                if memt_prev is not None:
                    emit_ctx(g - 1, memt_prev,
                             n_dve=(n_ctx_dve if g - 1 < NT - 2 else 8))

                # scan chain for this chunk.  z reads the score PSUM
                # directly; the 1e30 clamp keeps the reciprocal's bit-trick
                # seed in range even if exp(-w) saturates.
                z = sops.tile([R, TC], F32, tag="z")
                nc.scalar.activation(out=z, in_=pw[:, :], func=AF.Exp, scale=-1.0)
                den = sops.tile([R, TC], F32, tag="den")
                nc.vector.tensor_scalar(out=den, in0=z, scalar1=1e30,
                                        scalar2=1.0, op0=ALU.min, op1=ALU.add)
                rden = sops.tile([R, TC], F32, tag="rden")
                nc.vector.reciprocal_approx_fast(out=rden, in_=den)
                om = sops.tile([R, TC], F32, tag="om")
                nc.vector.tensor_tensor(out=om, in0=z, in1=rden, op=ALU.mult)
                p_t = sops.tile([R, TC], F32, tag="p")
                nc.vector.tensor_scalar(out=p_t, in0=om, scalar1=-1.0,
                                        scalar2=1.0, op0=ALU.mult, op1=ALU.add)
                nc.vector.tensor_tensor_scan(
                    out=Cx[:, gsl1], data0=om, data1=eps_sb[:, :],
                    initial=Cx[:, g * TC: g * TC + 1],
                    op0=ALU.mult, op1=ALU.max)
                Cc = sops.tile([R, TC], F32, tag="Cc")
                nc.vector.tensor_scalar(out=Cc, in0=Cx[:, gsl], scalar1=1e-10,
                                        scalar2=None, op0=ALU.max)
                rC = sops.tile([R, TC], F32, tag="rC")
                nc.vector.reciprocal_approx_fast(out=rC, in_=Cc)
                ratio = sops.tile([R, TC], F32, tag="ratio")
                nc.vector.tensor_tensor(out=ratio, in0=rC, in1=prev_sb[:, gsl],
                                        op=ALU.mult)
                nc.vector.tensor_tensor_scan(
                    out=Ssc[:, gsl1], data0=ratio, data1=ratio,
                    initial=Ssc[:, g * TC: g * TC + 1],
                    op0=ALU.add, op1=ALU.bypass)
                pc = sops.tile([R, TC], F32, tag="pc")
                nc.gpsimd.tensor_tensor(out=pc, in0=p_t, in1=Cx[:, gsl],
                                        op=ALU.mult)
                nc.vector.tensor_tensor(out=a_all[:, gsl],
                                        in0=pc, in1=Ssc[:, gsl1], op=ALU.mult)
                nc.scalar.dma_start(out=al_d[:, gsl], in_=a_all[:, gsl].bitcast(F32))

                memt_prev = memt
                if g + 1 < NT:
                    memt_cur = memt_next

            emit_ctx(NT - 1, memt_prev, n_dve=8)

            # ---- finalize contexts --------------------------------------
            for r in range(R):
                for ec in range(NE):
                    idx = r * NE + ec
                    nc.vector.tensor_reduce(
                        out=ctxrow[:, idx:idx + 1], in_=parts[idx][:, :],
                        axis=mybir.AxisListType.X, op=ALU.add)
                nc.scalar.dma_start(
                    out=ctx_d[r].rearrange("(ec p) -> p ec", p=128),
                    in_=ctxrow[:, r * NE:(r + 1) * NE])

    nc.compile()
    return nc


_NC_CACHE = {}


def _get_nc():
    key = "full"
    if key not in _NC_CACHE:
        _NC_CACHE[key] = build_nc(R=N_FULL // N_CORES, T=T_DIM, E=E_DIM,
                                  D=D_DIM, U=U_DIM)
    return _NC_CACHE[key]


def _make_eye4(R):
    eye = np.zeros((R, R * 128), dtype=np.float32)
    for r in range(R):
        eye[r, r * 128:(r + 1) * 128] = 1.0
    return eye


def _make_vblk(v, R):
    U = v.shape[0]
    NU = U // 128
    vblk = np.zeros((128, NU, R, R), dtype=np.float32)
    for uh in range(NU):
        for r in range(R):
            vblk[:, uh, r, r] = v[uh * 128:(uh + 1) * 128]
    return vblk


def kernel(queries, previous_alignments, memory, W_q, W_k, v):
    queries = np.ascontiguousarray(np.asarray(queries, dtype=np.float32))
    prev = np.ascontiguousarray(np.asarray(previous_alignments, dtype=np.float32))
    memory = np.asarray(memory, dtype=np.float32)
    W_q = np.ascontiguousarray(np.asarray(W_q, dtype=np.float32))
    W_k = np.ascontiguousarray(np.asarray(W_k, dtype=np.float32))
    v = np.ascontiguousarray(np.asarray(v, dtype=np.float32))

    memT = np.ascontiguousarray(memory.transpose(0, 2, 1))  # [N, E, T]
    R = N_FULL // N_CORES
    eye4 = _make_eye4(R)
    vblk = _make_vblk(v, R)

    nc = _get_nc()
    in_maps = []
    for c in range(N_CORES):
        sl = slice(c * R, (c + 1) * R)
        in_maps.append({
            "memT": memT[sl], "prev": prev[sl], "queries": queries[sl],
            "W_q": W_q, "W_k": W_k, "vblk": vblk, "eye4": eye4,
        })
    res = run_bass_kernel_spmd(nc, in_maps, core_ids=list(range(N_CORES)))
    contexts = np.concatenate([res.results[c]["contexts"] for c in range(N_CORES)], axis=0)
    alignments = np.concatenate([res.results[c]["alignments"] for c in range(N_CORES)], axis=0)
    return (contexts, alignments)
